# revision 18
# baseline (speedup 1.0000x reference)
"""Row-parallel cached-embedding (CachedParamMgr.prepare_ids) kernel for 8 trn2 cores.

Semantics (verified against the jax reference):
  rows = idx_map[ids] (identity), ids < C hit the warm cache, ids >= C miss.
  Misses are assigned cache slots by global stable LFU order of freq_cnter[:C]
  with slots holding currently-requested (hit) rows protected.
  Outputs: gpu_row_idxs[N] (assigned slot per id) and out[N, D] (the rows:
  cuda_weight[id] for hits, weight[id] for misses -- write-back never touches
  rows >= C, so the gathered miss rows are the original weight rows).

Distribution: the cache table and the miss-reachable weight tail are sharded
row-wise across the 8 cores; unique ids are routed to their owning core on the
host, each core gathers its rows with indirect DMA, and the host inverse-routes
the gathered rows into the full output.
"""

import numpy as np

E = 1_000_000
D = 128
C = 262_144
N = 16_384
M = 8                      # cores
GRP = 4                    # tiles per store group (one barrier sem each)
PAD = np.int32(2**31 - 1)  # > bounds_check => row skipped by indirect DMA
# (hit tiles, miss tiles, cache-shard rows, weight-shard rows) variants,
# smallest first; each tile is 128 rows. Shard row-counts are static compile
# shapes; the shard START is chosen per run (load-balanced contiguous ranges).
VARIANTS = [
    (5, 12, 45_056, 106_496),
    (7, 14, 131_072, 245_760),
    (16, 16, 262_144, 737_856),   # full-table shards: covers any distribution
]

_COMPILED = {}


def _install_axon_hooks_shim():
    """bass_utils imports antenv.axon_hooks when BASS_TRACE is set; provide it
    if the environment doesn't, backed by libaxon_pjrt.so when present."""
    import sys
    try:
        import antenv.axon_hooks  # noqa: F401
        return
    except ImportError:
        pass
    import types
    mod = types.ModuleType("antenv.axon_hooks")
    hook = None
    try:
        import contextlib
        import ctypes
        import os
        so = "/opt/axon/libaxon_pjrt.so"
        if os.path.exists(so):
            lib = ctypes.CDLL(so)
            if hasattr(lib, "axon_start_nrt_profile"):
                lib.axon_start_nrt_profile.argtypes = [
                    ctypes.POINTER(ctypes.c_int64),
                    ctypes.c_size_t,
                ]
                lib.axon_start_nrt_profile.restype = ctypes.c_int64
                lib.axon_stop_nrt_profile.argtypes = [ctypes.c_char_p]
                lib.axon_stop_nrt_profile.restype = ctypes.c_int64

                @contextlib.contextmanager
                def _hook(output_dir, device_ids):
                    import jax
                    jax.devices()
                    if device_ids:
                        ids = (ctypes.c_int64 * len(device_ids))(*device_ids)
                        rc = lib.axon_start_nrt_profile(ids, len(device_ids))
                    else:
                        rc = lib.axon_start_nrt_profile(None, 0)
                    if rc != 0:
                        raise RuntimeError(f"axon_start_nrt_profile rc={rc}")
                    try:
                        yield
                    finally:
                        lib.axon_stop_nrt_profile(str(output_dir).encode())

                hook = _hook
    except Exception:
        hook = None
    mod.get_axon_ntff_profile_hook = lambda: hook
    mod.set_axon_ntff_profile_hook = lambda h: None
    sys.modules["antenv.axon_hooks"] = mod


def _build(nt_h, nt_m, csh, wsh):
    """Build the SPMD bass program: per core, gather routed unique rows from
    its cache shard and weight-tail shard into a compact [(nt_h+nt_m)*128, D]
    output. Gathers (SWDGE indirect, one row per partition-descriptor) are
    issued back-to-back on gpsimd; stores drain per group of GRP tiles on the
    sync engine's HWDGE queue, overlapping later gathers."""
    key = (nt_h, nt_m, csh, wsh)
    if key in _COMPILED:
        return _COMPILED[key]
    import contextlib

    import concourse.bass as bass
    import concourse.mybir as mybir

    nt = nt_h + nt_m
    cap = nt * 128
    # groups of GRP tiles, but taper the tail (2, then 1) so the final
    # store chain waits on as little gather data as possible
    sizes = []
    rem = nt - 3
    while rem > 0:
        sizes.append(min(GRP, rem))
        rem -= GRP
    sizes += [2, 1]
    groups = []
    t = 0
    for w in sizes:
        groups.append((t, t + w))
        t += w
    assert t == nt

    nc = bass.Bass()
    ctab = nc.dram_tensor("ctab", [csh, D], mybir.dt.float32, kind="ExternalInput")
    wtab = nc.dram_tensor("wtab", [wsh, D], mybir.dt.float32, kind="ExternalInput")
    idx = nc.dram_tensor("idx", [128, nt], mybir.dt.int32, kind="ExternalInput")
    out = nc.dram_tensor("out", [cap, D], mybir.dt.float32, kind="ExternalOutput")

    idx_sb = nc.alloc_sbuf_tensor("idx_sb", [128, nt], mybir.dt.int32).ap()
    rows_sb = nc.alloc_sbuf_tensor("rows_sb", [128, nt * D], mybir.dt.float32).ap()

    with contextlib.ExitStack() as ctx:
        # gathers are sem-tracked before the stores run, so the SWDGE queue is
        # empty by kernel end -- skip GpSimd's expensive dge_drain
        block = ctx.enter_context(nc.Block(no_gpsimd_drain=True))
        isem = ctx.enter_context(nc.semaphore("isem"))
        gsems = [ctx.enter_context(nc.semaphore(f"gsem{g}")) for g in range(len(groups))]
        ssem = ctx.enter_context(nc.semaphore("ssem"))

        @block.gpsimd
        def _(g: bass.BassEngine):
            bound_c = g.to_reg(csh - 1)
            bound_w = g.to_reg(wsh - 1)
            g.wait_ge(isem, 16)
            for gi, (t0, t1) in enumerate(groups):
                for t in range(t0, t1):
                    g.indirect_dma_start(
                        out=rows_sb[:, t * D : (t + 1) * D],
                        out_offset=None,
                        in_=(ctab if t < nt_h else wtab)[:],
                        in_offset=bass.IndirectOffsetOnAxis(
                            ap=idx_sb[:, t : t + 1], axis=0
                        ),
                        bounds_check=bound_c if t < nt_h else bound_w,
                        oob_is_err=False,
                    ).then_inc(gsems[gi], 16)

        @block.scalar
        def _(sc: bass.BassEngine):
            # scalar's HWDGE queue is idle at kernel start; sync's has a
            # preamble drain that would delay the idx load
            sc.dma_start(out=idx_sb[:], in_=idx[:]).then_inc(isem, 16)

        @block.sync
        def _(s: bass.BassEngine):
            for gi, (t0, t1) in enumerate(groups):
                w = t1 - t0
                # full count: all 16 SDMA engines inced for every gather in
                # the group => the whole group's data is in SBUF
                s.wait_ge(gsems[gi], w * 16)
                s.dma_start(
                    out=out[t0 * 128 : t1 * 128, :].rearrange(
                        "(t p) d -> p t d", p=128
                    ),
                    in_=rows_sb[:, t0 * D : t1 * D].rearrange(
                        "p (t d) -> p t d", t=w
                    ),
                ).then_inc(ssem, 16)
            s.wait_ge(ssem, len(groups) * 16)

    _COMPILED[key] = nc
    return nc


def _index_plane(ids64, freq_cnter):
    """Host index plane: unique/rank + stable LFU slot assignment.
    Returns (u_all, inv_all, n_hit, gpu_u) where gpu_u[j] is the cache slot of
    the j-th sorted unique id."""
    u_all, inv_all = np.unique(ids64, return_inverse=True)
    n_hit = int(np.searchsorted(u_all, C))
    u_hit = u_all[:n_hit]
    u_miss = u_all[n_hit:]
    keys = freq_cnter[:C].astype(np.int64)
    keys[u_hit] = np.iinfo(np.int64).max  # protect slots of requested rows
    order = np.argsort(keys, kind="stable")
    slots = order[: len(u_miss)]
    gpu_u = np.concatenate([u_hit, slots])
    return u_all, inv_all, n_hit, gpu_u


def _host_fallback(ids, idx_map, cached_idx_map, inverted_cached_idx, freq_cnter,
                   weight, cuda_weight):
    """Faithful numpy port of the jax reference (general inputs)."""
    i64 = np.int64
    ids = ids.astype(i64)
    rows = idx_map.astype(i64)[ids]
    u = np.full(N, E, dtype=i64)
    uu = np.unique(rows)
    u[: len(uu)] = uu[:N]
    valid = u < E
    u_safe = np.where(valid, u, 0)
    slot_of_u = np.where(valid, inverted_cached_idx.astype(i64)[u_safe], -1)
    hit = valid & (slot_of_u >= 0)
    miss = valid & (slot_of_u < 0)

    protected = np.zeros(C, bool)
    hs = slot_of_u[hit]
    protected[hs[(hs >= 0) & (hs < C)]] = True
    cached_rows = cached_idx_map.astype(i64)
    keys = freq_cnter.astype(i64)[np.clip(cached_rows, 0, E - 1)]
    keys = np.where(cached_rows < 0, i64(-1), keys)
    keys = np.where(protected, np.iinfo(i64).max, keys)
    order = np.argsort(keys, kind="stable")

    rank = np.cumsum(miss) - 1
    slot_for = np.where(miss, order[np.clip(rank, 0, C - 1)], C)
    slot_for_safe = np.clip(slot_for, 0, C - 1)
    evicted_rows = cached_rows[slot_for_safe]

    wb_row = np.where(miss & (evicted_rows >= 0), evicted_rows, E)
    weight_new = weight.copy()
    mask = wb_row < E
    weight_new[wb_row[mask]] = cuda_weight[slot_for_safe[mask]]
    cuda_new = cuda_weight.copy()
    mask = (slot_for >= 0) & (slot_for < C)
    cuda_new[slot_for[mask]] = weight_new[u_safe[mask]]

    inv = inverted_cached_idx.astype(i64).copy()
    inv[wb_row[wb_row < E]] = -1
    tgt = np.where(miss, u, E)
    mask = tgt < E
    inv[tgt[mask]] = slot_for[mask]

    gpu_row_idxs = inv[rows]
    out = cuda_new[np.clip(gpu_row_idxs, 0, C - 1)]
    return gpu_row_idxs.astype(ids.dtype), out.astype(np.float32)


def _structure_ok(idx_map, cached_idx_map, inverted_cached_idx, ids64):
    if ids64.size != N or ids64.min() < 0 or ids64.max() >= E:
        return False
    ar_e = np.arange(E, dtype=idx_map.dtype)
    if not np.array_equal(idx_map, ar_e):
        return False
    if not np.array_equal(cached_idx_map, np.arange(C, dtype=cached_idx_map.dtype)):
        return False
    inv_exp = np.where(np.arange(E) < C, np.arange(E), -1).astype(
        inverted_cached_idx.dtype
    )
    return np.array_equal(inverted_cached_idx, inv_exp)


def kernel(ids, idx_map, cached_idx_map, inverted_cached_idx, freq_cnter,
           weight, cuda_weight):
    ids = np.asarray(ids)
    idx_map = np.asarray(idx_map)
    cached_idx_map = np.asarray(cached_idx_map)
    inverted_cached_idx = np.asarray(inverted_cached_idx)
    freq_cnter = np.asarray(freq_cnter)
    weight = np.asarray(weight, dtype=np.float32)
    cuda_weight = np.asarray(cuda_weight, dtype=np.float32)

    ids64 = ids.astype(np.int64)
    if not _structure_ok(idx_map, cached_idx_map, inverted_cached_idx, ids64):
        return _host_fallback(ids, idx_map, cached_idx_map, inverted_cached_idx,
                              freq_cnter, weight, cuda_weight)

    u_all, inv_all, n_hit, gpu_u = _index_plane(ids64, freq_cnter)
    u_hit = u_all[:n_hit]
    u_miss = u_all[n_hit:]
    n_miss = len(u_miss)

    # Load-balanced row-wise sharding: split the sorted unique hit/miss rows
    # into 8 near-equal contiguous chunks; core k's table shard is a fixed-size
    # window of cuda_weight/weight starting at its chunk's first row.
    h_off = np.round(np.linspace(0, n_hit, M + 1)).astype(np.int64)
    m_off = np.round(np.linspace(0, n_miss, M + 1)).astype(np.int64)
    pick = None
    for nt_h, nt_m, csh, wsh in VARIANTS:
        if (np.max(h_off[1:] - h_off[:-1]) > nt_h * 128
                or np.max(m_off[1:] - m_off[:-1]) > nt_m * 128):
            continue
        spans_ok = True
        for k in range(M):
            if h_off[k + 1] > h_off[k]:
                if u_hit[h_off[k + 1] - 1] - u_hit[h_off[k]] + 1 > csh:
                    spans_ok = False
            if m_off[k + 1] > m_off[k]:
                if u_miss[m_off[k + 1] - 1] - u_miss[m_off[k]] + 1 > wsh:
                    spans_ok = False
        if spans_ok:
            pick = (nt_h, nt_m, csh, wsh)
            break
    if pick is None:
        return _host_fallback(ids, idx_map, cached_idx_map, inverted_cached_idx,
                              freq_cnter, weight, cuda_weight)
    nt_h, nt_m, csh, wsh = pick
    hcap, nt = nt_h * 128, nt_h + nt_m

    _install_axon_hooks_shim()
    from concourse.bass_utils import run_bass_kernel_spmd

    nc = _build(nt_h, nt_m, csh, wsh)
    in_maps = []
    for k in range(M):
        hk = u_hit[h_off[k] : h_off[k + 1]]
        mk = u_miss[m_off[k] : m_off[k + 1]]
        hstart = min(int(hk[0]), C - csh) if len(hk) else 0
        mstart = min(int(mk[0]), E - wsh) if len(mk) else C
        idxmat = np.full((nt, 128), PAD, dtype=np.int32)
        idxmat.reshape(-1)[: len(hk)] = hk - hstart
        idxmat.reshape(-1)[hcap : hcap + len(mk)] = mk - mstart
        in_maps.append(
            {
                "ctab": cuda_weight[hstart : hstart + csh],
                "wtab": weight[mstart : mstart + wsh],
                "idx": np.ascontiguousarray(idxmat.T),
            }
        )

    res = run_bass_kernel_spmd(nc, in_maps, core_ids=list(range(M)))

    gath = np.empty((len(u_all), D), np.float32)
    for k in range(M):
        o = res.results[k]["out"]
        gath[h_off[k] : h_off[k + 1]] = o[: h_off[k + 1] - h_off[k]]
        gath[n_hit + m_off[k] : n_hit + m_off[k + 1]] = o[
            hcap : hcap + m_off[k + 1] - m_off[k]
        ]

    out = gath[inv_all]
    gpu_row_idxs = gpu_u[inv_all].astype(ids.dtype)
    return gpu_row_idxs, out


# revision 20
# speedup vs baseline: 1.0355x; 1.0355x over previous
"""Row-parallel cached-embedding (CachedParamMgr.prepare_ids) kernel for 8 trn2 cores.

Semantics (verified against the jax reference):
  rows = idx_map[ids] (identity), ids < C hit the warm cache, ids >= C miss.
  Misses are assigned cache slots by global stable LFU order of freq_cnter[:C]
  with slots holding currently-requested (hit) rows protected.
  Outputs: gpu_row_idxs[N] (assigned slot per id) and out[N, D] (the rows:
  cuda_weight[id] for hits, weight[id] for misses -- write-back never touches
  rows >= C, so the gathered miss rows are the original weight rows).

Distribution: the cache table and the miss-reachable weight tail are sharded
row-wise across the 8 cores; unique ids are routed to their owning core on the
host, each core gathers its rows with indirect DMA, and the host inverse-routes
the gathered rows into the full output.
"""

import numpy as np

E = 1_000_000
D = 128
C = 262_144
N = 16_384
M = 8                      # cores
GRP = 4                    # tiles per store group (one barrier sem each)
PAD = np.int32(2**31 - 1)  # > bounds_check => row skipped by indirect DMA
# (hit tiles, miss tiles, cache-shard rows, weight-shard rows) variants,
# smallest first; each tile is 128 rows. Shard row-counts are static compile
# shapes; the shard START is chosen per run (load-balanced contiguous ranges).
VARIANTS = [
    (5, 12, 45_056, 106_496),
    (7, 14, 131_072, 245_760),
    (16, 16, 262_144, 737_856),   # full-table shards: covers any distribution
]

_COMPILED = {}


def _install_axon_hooks_shim():
    """bass_utils imports antenv.axon_hooks when BASS_TRACE is set; provide it
    if the environment doesn't, backed by libaxon_pjrt.so when present."""
    import sys
    try:
        import antenv.axon_hooks  # noqa: F401
        return
    except ImportError:
        pass
    import types
    mod = types.ModuleType("antenv.axon_hooks")
    hook = None
    try:
        import contextlib
        import ctypes
        import os
        so = "/opt/axon/libaxon_pjrt.so"
        if os.path.exists(so):
            lib = ctypes.CDLL(so)
            if hasattr(lib, "axon_start_nrt_profile"):
                lib.axon_start_nrt_profile.argtypes = [
                    ctypes.POINTER(ctypes.c_int64),
                    ctypes.c_size_t,
                ]
                lib.axon_start_nrt_profile.restype = ctypes.c_int64
                lib.axon_stop_nrt_profile.argtypes = [ctypes.c_char_p]
                lib.axon_stop_nrt_profile.restype = ctypes.c_int64

                @contextlib.contextmanager
                def _hook(output_dir, device_ids):
                    import jax
                    jax.devices()
                    if device_ids:
                        ids = (ctypes.c_int64 * len(device_ids))(*device_ids)
                        rc = lib.axon_start_nrt_profile(ids, len(device_ids))
                    else:
                        rc = lib.axon_start_nrt_profile(None, 0)
                    if rc != 0:
                        raise RuntimeError(f"axon_start_nrt_profile rc={rc}")
                    try:
                        yield
                    finally:
                        lib.axon_stop_nrt_profile(str(output_dir).encode())

                hook = _hook
    except Exception:
        hook = None
    mod.get_axon_ntff_profile_hook = lambda: hook
    mod.set_axon_ntff_profile_hook = lambda h: None
    sys.modules["antenv.axon_hooks"] = mod


def _build(nt_h, nt_m, csh, wsh):
    """Build the SPMD bass program: per core, gather routed unique rows from
    its cache shard and weight-tail shard into a compact [(nt_h+nt_m)*128, D]
    output. Gathers (SWDGE indirect, one row per partition-descriptor) are
    issued back-to-back on gpsimd; stores drain per group of GRP tiles on the
    sync engine's HWDGE queue, overlapping later gathers."""
    key = (nt_h, nt_m, csh, wsh)
    if key in _COMPILED:
        return _COMPILED[key]
    import contextlib

    import concourse.bass as bass
    import concourse.mybir as mybir

    nt = nt_h + nt_m
    cap = nt * 128
    # groups of GRP tiles, but taper the tail (2, then 1) so the final
    # store chain waits on as little gather data as possible
    sizes = []
    rem = nt - 3
    while rem > 0:
        sizes.append(min(GRP, rem))
        rem -= GRP
    sizes += [2, 1]
    groups = []
    t = 0
    for w in sizes:
        groups.append((t, t + w))
        t += w
    assert t == nt

    nc = bass.Bass()
    ctab = nc.dram_tensor("ctab", [csh, D], mybir.dt.float32, kind="ExternalInput")
    wtab = nc.dram_tensor("wtab", [wsh, D], mybir.dt.float32, kind="ExternalInput")
    idx = nc.dram_tensor("idx", [128, nt], mybir.dt.int32, kind="ExternalInput")
    out = nc.dram_tensor("out", [cap, D], mybir.dt.float32, kind="ExternalOutput")

    idx_sb = nc.alloc_sbuf_tensor("idx_sb", [128, nt], mybir.dt.int32).ap()
    rows_sb = nc.alloc_sbuf_tensor("rows_sb", [128, nt * D], mybir.dt.float32).ap()

    with contextlib.ExitStack() as ctx:
        # gathers are sem-tracked before the stores run, so the SWDGE queue is
        # empty by kernel end -- skip GpSimd's expensive dge_drain
        block = ctx.enter_context(nc.Block(no_gpsimd_drain=True))
        isem = ctx.enter_context(nc.semaphore("isem"))
        gsems = [ctx.enter_context(nc.semaphore(f"gsem{g}")) for g in range(len(groups))]
        ssem = ctx.enter_context(nc.semaphore("ssem"))

        @block.gpsimd
        def _(g: bass.BassEngine):
            bound_c = g.to_reg(csh - 1)
            bound_w = g.to_reg(wsh - 1)
            g.wait_ge(isem, 16)
            for gi, (t0, t1) in enumerate(groups):
                for t in range(t0, t1):
                    g.indirect_dma_start(
                        out=rows_sb[:, t * D : (t + 1) * D],
                        out_offset=None,
                        in_=(ctab if t < nt_h else wtab)[:],
                        in_offset=bass.IndirectOffsetOnAxis(
                            ap=idx_sb[:, t : t + 1], axis=0
                        ),
                        bounds_check=bound_c if t < nt_h else bound_w,
                        oob_is_err=False,
                    ).then_inc(gsems[gi], 16)

        @block.scalar
        def _(sc: bass.BassEngine):
            # scalar's HWDGE queue is idle at kernel start; sync's has a
            # preamble drain that would delay the idx load
            sc.dma_start(out=idx_sb[:], in_=idx[:]).then_inc(isem, 16)

        @block.sync
        def _(s: bass.BassEngine):
            for gi, (t0, t1) in enumerate(groups):
                w = t1 - t0
                # full count: all 16 SDMA engines inced for every gather in
                # the group => the whole group's data is in SBUF
                s.wait_ge(gsems[gi], w * 16)
                s.dma_start(
                    out=out[t0 * 128 : t1 * 128, :].rearrange(
                        "(t p) d -> p t d", p=128
                    ),
                    in_=rows_sb[:, t0 * D : t1 * D].rearrange(
                        "p (t d) -> p t d", t=w
                    ),
                ).then_inc(ssem, 16)
            s.wait_ge(ssem, len(groups) * 16)

    _COMPILED[key] = nc
    return nc


def _index_plane(ids64, freq_cnter):
    """Host index plane: unique/rank + stable LFU slot assignment.
    Returns (u_all, inv_all, n_hit, gpu_u) where gpu_u[j] is the cache slot of
    the j-th sorted unique id."""
    u_all, inv_all = np.unique(ids64, return_inverse=True)
    n_hit = int(np.searchsorted(u_all, C))
    u_hit = u_all[:n_hit]
    u_miss = u_all[n_hit:]
    keys = freq_cnter[:C].astype(np.int64)
    keys[u_hit] = np.iinfo(np.int64).max  # protect slots of requested rows
    order = np.argsort(keys, kind="stable")
    slots = order[: len(u_miss)]
    gpu_u = np.concatenate([u_hit, slots])
    return u_all, inv_all, n_hit, gpu_u


def _host_fallback(ids, idx_map, cached_idx_map, inverted_cached_idx, freq_cnter,
                   weight, cuda_weight):
    """Faithful numpy port of the jax reference (general inputs)."""
    i64 = np.int64
    out_dtype = ids.dtype
    ids = ids.astype(i64)
    rows = idx_map.astype(i64)[ids]
    u = np.full(N, E, dtype=i64)
    uu = np.unique(rows)
    u[: len(uu)] = uu[:N]
    valid = u < E
    u_safe = np.where(valid, u, 0)
    slot_of_u = np.where(valid, inverted_cached_idx.astype(i64)[u_safe], -1)
    hit = valid & (slot_of_u >= 0)
    miss = valid & (slot_of_u < 0)

    protected = np.zeros(C, bool)
    hs = slot_of_u[hit]
    protected[hs[(hs >= 0) & (hs < C)]] = True
    cached_rows = cached_idx_map.astype(i64)
    keys = freq_cnter.astype(i64)[np.clip(cached_rows, 0, E - 1)]
    keys = np.where(cached_rows < 0, i64(-1), keys)
    keys = np.where(protected, np.iinfo(i64).max, keys)
    order = np.argsort(keys, kind="stable")

    rank = np.cumsum(miss) - 1
    slot_for = np.where(miss, order[np.clip(rank, 0, C - 1)], C)
    slot_for_safe = np.clip(slot_for, 0, C - 1)
    evicted_rows = cached_rows[slot_for_safe]

    wb_row = np.where(miss & (evicted_rows >= 0), evicted_rows, E)
    weight_new = weight.copy()
    mask = wb_row < E
    weight_new[wb_row[mask]] = cuda_weight[slot_for_safe[mask]]
    cuda_new = cuda_weight.copy()
    mask = (slot_for >= 0) & (slot_for < C)
    cuda_new[slot_for[mask]] = weight_new[u_safe[mask]]

    inv = inverted_cached_idx.astype(i64).copy()
    inv[wb_row[wb_row < E]] = -1
    tgt = np.where(miss, u, E)
    mask = tgt < E
    inv[tgt[mask]] = slot_for[mask]

    gpu_row_idxs = inv[rows]
    # match jax indexing: negative indices wrap, overlarge clamp
    gi = np.where(gpu_row_idxs < 0, gpu_row_idxs + C, gpu_row_idxs)
    out = cuda_new[np.clip(gi, 0, C - 1)]
    return gpu_row_idxs.astype(out_dtype), out.astype(np.float32)


def _structure_ok(idx_map, cached_idx_map, inverted_cached_idx, ids64):
    if ids64.size != N or ids64.min() < 0 or ids64.max() >= E:
        return False
    ar_e = np.arange(E, dtype=idx_map.dtype)
    if not np.array_equal(idx_map, ar_e):
        return False
    if not np.array_equal(cached_idx_map, np.arange(C, dtype=cached_idx_map.dtype)):
        return False
    inv_exp = np.where(np.arange(E) < C, np.arange(E), -1).astype(
        inverted_cached_idx.dtype
    )
    return np.array_equal(inverted_cached_idx, inv_exp)


def kernel(ids, idx_map, cached_idx_map, inverted_cached_idx, freq_cnter,
           weight, cuda_weight):
    ids = np.asarray(ids)
    idx_map = np.asarray(idx_map)
    cached_idx_map = np.asarray(cached_idx_map)
    inverted_cached_idx = np.asarray(inverted_cached_idx)
    freq_cnter = np.asarray(freq_cnter)
    weight = np.asarray(weight, dtype=np.float32)
    cuda_weight = np.asarray(cuda_weight, dtype=np.float32)

    ids64 = ids.astype(np.int64)
    if not _structure_ok(idx_map, cached_idx_map, inverted_cached_idx, ids64):
        return _host_fallback(ids, idx_map, cached_idx_map, inverted_cached_idx,
                              freq_cnter, weight, cuda_weight)

    u_all, inv_all, n_hit, gpu_u = _index_plane(ids64, freq_cnter)
    u_hit = u_all[:n_hit]
    u_miss = u_all[n_hit:]
    n_miss = len(u_miss)

    # Load-balanced row-wise sharding: split the sorted unique hit/miss rows
    # into 8 near-equal contiguous chunks; core k's table shard is a fixed-size
    # window of cuda_weight/weight starting at its chunk's first row.
    h_off = np.round(np.linspace(0, n_hit, M + 1)).astype(np.int64)
    m_off = np.round(np.linspace(0, n_miss, M + 1)).astype(np.int64)
    pick = None
    for nt_h, nt_m, csh, wsh in VARIANTS:
        if (np.max(h_off[1:] - h_off[:-1]) > nt_h * 128
                or np.max(m_off[1:] - m_off[:-1]) > nt_m * 128):
            continue
        spans_ok = True
        for k in range(M):
            if h_off[k + 1] > h_off[k]:
                if u_hit[h_off[k + 1] - 1] - u_hit[h_off[k]] + 1 > csh:
                    spans_ok = False
            if m_off[k + 1] > m_off[k]:
                if u_miss[m_off[k + 1] - 1] - u_miss[m_off[k]] + 1 > wsh:
                    spans_ok = False
        if spans_ok:
            pick = (nt_h, nt_m, csh, wsh)
            break
    if pick is None:
        return _host_fallback(ids, idx_map, cached_idx_map, inverted_cached_idx,
                              freq_cnter, weight, cuda_weight)
    nt_h, nt_m, csh, wsh = pick
    hcap, nt = nt_h * 128, nt_h + nt_m

    _install_axon_hooks_shim()
    from concourse.bass_utils import run_bass_kernel_spmd

    nc = _build(nt_h, nt_m, csh, wsh)
    in_maps = []
    for k in range(M):
        hk = u_hit[h_off[k] : h_off[k + 1]]
        mk = u_miss[m_off[k] : m_off[k + 1]]
        hstart = min(int(hk[0]), C - csh) if len(hk) else 0
        mstart = min(int(mk[0]), E - wsh) if len(mk) else C
        idxmat = np.full((nt, 128), PAD, dtype=np.int32)
        idxmat.reshape(-1)[: len(hk)] = hk - hstart
        idxmat.reshape(-1)[hcap : hcap + len(mk)] = mk - mstart
        in_maps.append(
            {
                "ctab": cuda_weight[hstart : hstart + csh],
                "wtab": weight[mstart : mstart + wsh],
                "idx": np.ascontiguousarray(idxmat.T),
            }
        )

    res = run_bass_kernel_spmd(nc, in_maps, core_ids=list(range(M)))

    gath = np.empty((len(u_all), D), np.float32)
    for k in range(M):
        o = res.results[k]["out"]
        gath[h_off[k] : h_off[k + 1]] = o[: h_off[k + 1] - h_off[k]]
        gath[n_hit + m_off[k] : n_hit + m_off[k + 1]] = o[
            hcap : hcap + m_off[k + 1] - m_off[k]
        ]

    out = gath[inv_all]
    gpu_row_idxs = gpu_u[inv_all].astype(ids.dtype)
    return gpu_row_idxs, out


# revision 23
# speedup vs baseline: 1.0573x; 1.0210x over previous
"""Row-parallel cached-embedding (CachedParamMgr.prepare_ids) kernel for 8 trn2 cores.

Semantics (verified against the jax reference):
  rows = idx_map[ids] (identity), ids < C hit the warm cache, ids >= C miss.
  Misses are assigned cache slots by global stable LFU order of freq_cnter[:C]
  with slots holding currently-requested (hit) rows protected.
  Outputs: gpu_row_idxs[N] (assigned slot per id) and out[N, D] (the rows:
  cuda_weight[id] for hits, weight[id] for misses -- write-back never touches
  rows >= C, so the gathered miss rows are the original weight rows).

Distribution: the cache table and the miss-reachable weight tail are sharded
row-wise across the 8 cores; unique ids are routed to their owning core on the
host, each core gathers its rows with indirect DMA, and the host inverse-routes
the gathered rows into the full output.
"""

import numpy as np

E = 1_000_000
D = 128
C = 262_144
N = 16_384
M = 8                      # cores
GRP = 4                    # tiles per store group (one barrier sem each)
# (hit tiles, miss tiles, cache-shard rows, weight-shard rows) variants,
# smallest first; each tile is 128 rows. Shard row-counts are static compile
# shapes; the shard START is chosen per run (load-balanced contiguous ranges).
VARIANTS = [
    (5, 12, 45_056, 106_496),
    (7, 14, 131_072, 245_760),
    (16, 16, 262_144, 737_856),   # full-table shards: covers any distribution
]

_COMPILED = {}


def _install_axon_hooks_shim():
    """bass_utils imports antenv.axon_hooks when BASS_TRACE is set; provide it
    if the environment doesn't, backed by libaxon_pjrt.so when present."""
    import sys
    try:
        import antenv.axon_hooks  # noqa: F401
        return
    except ImportError:
        pass
    import types
    mod = types.ModuleType("antenv.axon_hooks")
    hook = None
    try:
        import contextlib
        import ctypes
        import os
        so = "/opt/axon/libaxon_pjrt.so"
        if os.path.exists(so):
            lib = ctypes.CDLL(so)
            if hasattr(lib, "axon_start_nrt_profile"):
                lib.axon_start_nrt_profile.argtypes = [
                    ctypes.POINTER(ctypes.c_int64),
                    ctypes.c_size_t,
                ]
                lib.axon_start_nrt_profile.restype = ctypes.c_int64
                lib.axon_stop_nrt_profile.argtypes = [ctypes.c_char_p]
                lib.axon_stop_nrt_profile.restype = ctypes.c_int64

                @contextlib.contextmanager
                def _hook(output_dir, device_ids):
                    import jax
                    jax.devices()
                    if device_ids:
                        ids = (ctypes.c_int64 * len(device_ids))(*device_ids)
                        rc = lib.axon_start_nrt_profile(ids, len(device_ids))
                    else:
                        rc = lib.axon_start_nrt_profile(None, 0)
                    if rc != 0:
                        raise RuntimeError(f"axon_start_nrt_profile rc={rc}")
                    try:
                        yield
                    finally:
                        lib.axon_stop_nrt_profile(str(output_dir).encode())

                hook = _hook
    except Exception:
        hook = None
    mod.get_axon_ntff_profile_hook = lambda: hook
    mod.set_axon_ntff_profile_hook = lambda h: None
    sys.modules["antenv.axon_hooks"] = mod


def _build(nt_h, nt_m, csh, wsh):
    """Build the SPMD bass program: per core, gather routed unique rows from
    its cache shard and weight-tail shard into a compact [(nt_h+nt_m)*128, D]
    output. Gathers (SWDGE indirect, one row per partition-descriptor) are
    issued back-to-back on gpsimd; stores drain per group of GRP tiles on the
    sync engine's HWDGE queue, overlapping later gathers."""
    key = (nt_h, nt_m, csh, wsh)
    if key in _COMPILED:
        return _COMPILED[key]
    import contextlib

    import concourse.bass as bass
    import concourse.mybir as mybir

    nt = nt_h + nt_m
    cap = nt * 128
    # groups of GRP tiles, but taper the tail (2, then 1) so the final
    # store chain waits on as little gather data as possible
    sizes = []
    rem = nt - 3
    while rem > 0:
        sizes.append(min(GRP, rem))
        rem -= GRP
    sizes += [2, 1]
    groups = []
    t = 0
    for w in sizes:
        groups.append((t, t + w))
        t += w
    assert t == nt

    nc = bass.Bass()
    ctab = nc.dram_tensor("ctab", [csh, D], mybir.dt.float32, kind="ExternalInput")
    wtab = nc.dram_tensor("wtab", [wsh, D], mybir.dt.float32, kind="ExternalInput")
    idx = nc.dram_tensor("idx", [128, nt], mybir.dt.int32, kind="ExternalInput")
    out = nc.dram_tensor("out", [cap, D], mybir.dt.float32, kind="ExternalOutput")

    idx_sb = nc.alloc_sbuf_tensor("idx_sb", [128, nt], mybir.dt.int32).ap()
    rows_sb = nc.alloc_sbuf_tensor("rows_sb", [128, nt * D], mybir.dt.float32).ap()

    with contextlib.ExitStack() as ctx:
        # gathers are sem-tracked before the stores run, so the SWDGE queue is
        # empty by kernel end -- skip GpSimd's expensive dge_drain
        block = ctx.enter_context(nc.Block(no_gpsimd_drain=True))
        isem = ctx.enter_context(nc.semaphore("isem"))
        gsems = [ctx.enter_context(nc.semaphore(f"gsem{g}")) for g in range(len(groups))]
        ssem = ctx.enter_context(nc.semaphore("ssem"))

        @block.gpsimd
        def _(g: bass.BassEngine):
            # no bounds_check: the ucode bounds path costs ~50ns/instruction,
            # so pad lanes carry index 0 (a valid row; gathered and ignored)
            g.wait_ge(isem, 16)
            for gi, (t0, t1) in enumerate(groups):
                for t in range(t0, t1):
                    g.indirect_dma_start(
                        out=rows_sb[:, t * D : (t + 1) * D],
                        out_offset=None,
                        in_=(ctab if t < nt_h else wtab)[:],
                        in_offset=bass.IndirectOffsetOnAxis(
                            ap=idx_sb[:, t : t + 1], axis=0
                        ),
                    ).then_inc(gsems[gi], 16)

        @block.scalar
        def _(sc: bass.BassEngine):
            # scalar's HWDGE queue is idle at kernel start; sync's has a
            # preamble drain that would delay the idx load
            sc.dma_start(out=idx_sb[:], in_=idx[:]).then_inc(isem, 16)

        @block.sync
        def _(s: bass.BassEngine):
            for gi, (t0, t1) in enumerate(groups):
                w = t1 - t0
                # full count: all 16 SDMA engines inced for every gather in
                # the group => the whole group's data is in SBUF
                s.wait_ge(gsems[gi], w * 16)
                s.dma_start(
                    out=out[t0 * 128 : t1 * 128, :].rearrange(
                        "(t p) d -> p t d", p=128
                    ),
                    in_=rows_sb[:, t0 * D : t1 * D].rearrange(
                        "p (t d) -> p t d", t=w
                    ),
                ).then_inc(ssem, 16)
            s.wait_ge(ssem, len(groups) * 16)

    _COMPILED[key] = nc
    return nc


def _index_plane(ids64, freq_cnter):
    """Host index plane: unique/rank + stable LFU slot assignment.
    Returns (u_all, inv_all, n_hit, gpu_u) where gpu_u[j] is the cache slot of
    the j-th sorted unique id."""
    u_all, inv_all = np.unique(ids64, return_inverse=True)
    n_hit = int(np.searchsorted(u_all, C))
    u_hit = u_all[:n_hit]
    u_miss = u_all[n_hit:]
    keys = freq_cnter[:C].astype(np.int64)
    keys[u_hit] = np.iinfo(np.int64).max  # protect slots of requested rows
    order = np.argsort(keys, kind="stable")
    slots = order[: len(u_miss)]
    gpu_u = np.concatenate([u_hit, slots])
    return u_all, inv_all, n_hit, gpu_u


def _host_fallback(ids, idx_map, cached_idx_map, inverted_cached_idx, freq_cnter,
                   weight, cuda_weight):
    """Faithful numpy port of the jax reference (general inputs)."""
    i64 = np.int64
    out_dtype = ids.dtype
    ids = ids.astype(i64)
    rows = idx_map.astype(i64)[ids]
    u = np.full(N, E, dtype=i64)
    uu = np.unique(rows)
    u[: len(uu)] = uu[:N]
    valid = u < E
    u_safe = np.where(valid, u, 0)
    slot_of_u = np.where(valid, inverted_cached_idx.astype(i64)[u_safe], -1)
    hit = valid & (slot_of_u >= 0)
    miss = valid & (slot_of_u < 0)

    protected = np.zeros(C, bool)
    hs = slot_of_u[hit]
    protected[hs[(hs >= 0) & (hs < C)]] = True
    cached_rows = cached_idx_map.astype(i64)
    keys = freq_cnter.astype(i64)[np.clip(cached_rows, 0, E - 1)]
    keys = np.where(cached_rows < 0, i64(-1), keys)
    keys = np.where(protected, np.iinfo(i64).max, keys)
    order = np.argsort(keys, kind="stable")

    rank = np.cumsum(miss) - 1
    slot_for = np.where(miss, order[np.clip(rank, 0, C - 1)], C)
    slot_for_safe = np.clip(slot_for, 0, C - 1)
    evicted_rows = cached_rows[slot_for_safe]

    wb_row = np.where(miss & (evicted_rows >= 0), evicted_rows, E)
    weight_new = weight.copy()
    mask = wb_row < E
    weight_new[wb_row[mask]] = cuda_weight[slot_for_safe[mask]]
    cuda_new = cuda_weight.copy()
    mask = (slot_for >= 0) & (slot_for < C)
    cuda_new[slot_for[mask]] = weight_new[u_safe[mask]]

    inv = inverted_cached_idx.astype(i64).copy()
    inv[wb_row[wb_row < E]] = -1
    tgt = np.where(miss, u, E)
    mask = tgt < E
    inv[tgt[mask]] = slot_for[mask]

    gpu_row_idxs = inv[rows]
    # match jax indexing: negative indices wrap, overlarge clamp
    gi = np.where(gpu_row_idxs < 0, gpu_row_idxs + C, gpu_row_idxs)
    out = cuda_new[np.clip(gi, 0, C - 1)]
    return gpu_row_idxs.astype(out_dtype), out.astype(np.float32)


def _structure_ok(idx_map, cached_idx_map, inverted_cached_idx, ids64):
    if ids64.size != N or ids64.min() < 0 or ids64.max() >= E:
        return False
    ar_e = np.arange(E, dtype=idx_map.dtype)
    if not np.array_equal(idx_map, ar_e):
        return False
    if not np.array_equal(cached_idx_map, np.arange(C, dtype=cached_idx_map.dtype)):
        return False
    inv_exp = np.where(np.arange(E) < C, np.arange(E), -1).astype(
        inverted_cached_idx.dtype
    )
    return np.array_equal(inverted_cached_idx, inv_exp)


def kernel(ids, idx_map, cached_idx_map, inverted_cached_idx, freq_cnter,
           weight, cuda_weight):
    ids = np.asarray(ids)
    idx_map = np.asarray(idx_map)
    cached_idx_map = np.asarray(cached_idx_map)
    inverted_cached_idx = np.asarray(inverted_cached_idx)
    freq_cnter = np.asarray(freq_cnter)
    weight = np.asarray(weight, dtype=np.float32)
    cuda_weight = np.asarray(cuda_weight, dtype=np.float32)

    ids64 = ids.astype(np.int64)
    if not _structure_ok(idx_map, cached_idx_map, inverted_cached_idx, ids64):
        return _host_fallback(ids, idx_map, cached_idx_map, inverted_cached_idx,
                              freq_cnter, weight, cuda_weight)

    u_all, inv_all, n_hit, gpu_u = _index_plane(ids64, freq_cnter)
    u_hit = u_all[:n_hit]
    u_miss = u_all[n_hit:]
    n_miss = len(u_miss)

    # Load-balanced row-wise sharding: split the sorted unique hit/miss rows
    # into 8 near-equal contiguous chunks; core k's table shard is a fixed-size
    # window of cuda_weight/weight starting at its chunk's first row.
    h_off = np.round(np.linspace(0, n_hit, M + 1)).astype(np.int64)
    m_off = np.round(np.linspace(0, n_miss, M + 1)).astype(np.int64)
    pick = None
    for nt_h, nt_m, csh, wsh in VARIANTS:
        if (np.max(h_off[1:] - h_off[:-1]) > nt_h * 128
                or np.max(m_off[1:] - m_off[:-1]) > nt_m * 128):
            continue
        spans_ok = True
        for k in range(M):
            if h_off[k + 1] > h_off[k]:
                if u_hit[h_off[k + 1] - 1] - u_hit[h_off[k]] + 1 > csh:
                    spans_ok = False
            if m_off[k + 1] > m_off[k]:
                if u_miss[m_off[k + 1] - 1] - u_miss[m_off[k]] + 1 > wsh:
                    spans_ok = False
        if spans_ok:
            pick = (nt_h, nt_m, csh, wsh)
            break
    if pick is None:
        return _host_fallback(ids, idx_map, cached_idx_map, inverted_cached_idx,
                              freq_cnter, weight, cuda_weight)
    nt_h, nt_m, csh, wsh = pick
    hcap, nt = nt_h * 128, nt_h + nt_m

    _install_axon_hooks_shim()
    from concourse.bass_utils import run_bass_kernel_spmd

    nc = _build(nt_h, nt_m, csh, wsh)
    in_maps = []
    for k in range(M):
        hk = u_hit[h_off[k] : h_off[k + 1]]
        mk = u_miss[m_off[k] : m_off[k + 1]]
        hstart = min(int(hk[0]), C - csh) if len(hk) else 0
        mstart = min(int(mk[0]), E - wsh) if len(mk) else C
        idxmat = np.zeros((nt, 128), dtype=np.int32)
        idxmat.reshape(-1)[: len(hk)] = hk - hstart
        idxmat.reshape(-1)[hcap : hcap + len(mk)] = mk - mstart
        in_maps.append(
            {
                "ctab": cuda_weight[hstart : hstart + csh],
                "wtab": weight[mstart : mstart + wsh],
                "idx": np.ascontiguousarray(idxmat.T),
            }
        )

    res = run_bass_kernel_spmd(nc, in_maps, core_ids=list(range(M)))

    gath = np.empty((len(u_all), D), np.float32)
    for k in range(M):
        o = res.results[k]["out"]
        gath[h_off[k] : h_off[k + 1]] = o[: h_off[k + 1] - h_off[k]]
        gath[n_hit + m_off[k] : n_hit + m_off[k + 1]] = o[
            hcap : hcap + m_off[k + 1] - m_off[k]
        ]

    out = gath[inv_all]
    gpu_row_idxs = gpu_u[inv_all].astype(ids.dtype)
    return gpu_row_idxs, out
